# revision 7
# baseline (speedup 1.0000x reference)
"""DeepSet encoder (phi MLP -> sum/max pool -> rho MLP) as a Trainium2 Bass kernel.

Sharding: data-parallel over the batch dim. 64 samples -> 8 cores x 8 samples.
Weights are replicated on every core; no cross-core communication.

On-chip layout is feature-major ("transposed"): activations live as
[feature_partition, set_free] tiles so that
  - matmul contraction (over features) is on the partition dim,
  - the bias is a per-partition scalar (free on ScalarE's activation op),
  - sum/max pooling over the set dim is a free-axis reduction
    (sum comes for free via activation's accum_out).

BOTH phi matmuls run in fp8e4m3 with DoubleRow (2 fp8 rows per PE pass ->
half the matmul passes of bf16). fp8's weight-quantization error would
normally be the accuracy killer for phi2: sum-pooling adds 512 h2 rows
whose shared error term h1_mean . dW2 is coherent across the set, so the
~2.5% RMS weight error passes straight through to the output. The fix is
host-side only: W2 is quantized with error *diffusion* down each column --
each weight rounds to the fp8 neighbor that cancels the mu-weighted
running error sum (mu = E[h1], closed form from W1/b1 since x ~ N(0,1)).
That zeroes the coherent component and restores ~bf16-level end-to-end
error at fp8 speed.

Pooling epilogues are spread over three engines so the PE stays the
bottleneck: ScalarE does relu+bias with the sum-pool via its accumulator,
VectorE max-reduces the raw PSUM pre-activations (max_n relu(z_n + b) ==
relu(max_n z_n + b), so no h2 tensor is ever materialized), and the Pool
engine (gpsimd) evacuates phi1's relu into fp8 h1 tiles.

Self-contained: only relies on the system-installed concourse/bass stack.
"""

import math
import sys

import numpy as np

for _p in ("/opt/trn_rl_repo",):
    if _p not in sys.path:
        sys.path.insert(0, _p)

import ml_dtypes  # noqa: E402

import concourse.bass as bass  # noqa: E402,F401
import concourse.mybir as mybir  # noqa: E402
import concourse.tile as tile  # noqa: E402
from concourse import bacc  # noqa: E402
from concourse.bass_utils import run_bass_kernel_spmd  # noqa: E402

# 16-bit compute dtype for the tiny rho MLP: fp16 runs the PE at the same
# 1 cycle/row as bf16 but carries 10 mantissa bits instead of 8.
FP16 = mybir.dt.float16
FP32 = mybir.dt.float32
NP_FP16 = np.float16
FP8 = mybir.dt.float8e4
NP_FP8 = ml_dtypes.float8_e4m3
DOUBLE_ROW = mybir.MatmulPerfMode.DoubleRow

B, N, D_IN, D_H = 64, 512, 512, 1024
N_CORES = 8
BL = B // N_CORES  # samples per core
P = 128
K1 = D_IN // P  # phi1 feature tiles (4)
K2 = D_H // P  # phi2/rho2 contraction tiles & D_H output tiles (8)
KR1 = 2 * D_H // P  # rho1 contraction tiles (16)
KK1 = D_IN // 256  # phi1 DoubleRow chunks (2)
KK2 = D_H // 256  # phi2 DoubleRow chunks (4)

RELU = mybir.ActivationFunctionType.Relu
AX_X = mybir.AxisListType.X
OP_ADD = mybir.AluOpType.add
OP_MAX = mybir.AluOpType.max


def build_program() -> bacc.Bacc:
    nc = bacc.Bacc("TRN2", target_bir_lowering=False, debug=False, num_devices=N_CORES)

    # all staged host-side into the exact SBUF tile layouts so every DMA is
    # contiguous per partition (large descriptor runs):
    #   xt[b, p, kk, j, n] = x[b, n, kk*256 + j*128 + p]   (fp8, DoubleRow pairs)
    #   w1[p, kk, j, h] = W1[kk*256 + j*128 + p, h]        (fp8)
    #   w2[p, kk, j, h] = W2[kk*256 + j*128 + p, h]        (fp8, error-diffused)
    #   wr*[p, ko, h] = W[ko*128+p, h]                     (fp16)
    xt_d = nc.dram_tensor("xt", [BL, P, KK1, 2, N], FP8, kind="ExternalInput").ap()
    w1_d = nc.dram_tensor("w1", [P, KK1, 2, D_H], FP8, kind="ExternalInput").ap()
    w2_d = nc.dram_tensor("w2", [P, KK2, 2, D_H], FP8, kind="ExternalInput").ap()
    wr1_d = nc.dram_tensor("wr1", [P, KR1, D_H], FP16, kind="ExternalInput").ap()
    wr2_d = nc.dram_tensor("wr2", [P, K2, D_H], FP16, kind="ExternalInput").ap()
    # biases staged on host as [P, n_tiles]: b_sb[p, m] = b[m*128 + p]
    b1_d = nc.dram_tensor("b1", [P, K2], FP32, kind="ExternalInput").ap()
    b2_d = nc.dram_tensor("b2", [P, K2], FP32, kind="ExternalInput").ap()
    br1_d = nc.dram_tensor("br1", [P, K2], FP32, kind="ExternalInput").ap()
    br2_d = nc.dram_tensor("br2", [P, K2], FP32, kind="ExternalInput").ap()
    # out[p, m, s] = r2[m*128 + p, s]  (feature-major, host transposes back)
    out_d = nc.dram_tensor("out", [P, K2, BL], FP32, kind="ExternalOutput").ap()

    with tile.TileContext(nc) as tc:
        with (
            tc.tile_pool(name="const", bufs=1) as cpool,
            tc.tile_pool(name="xt", bufs=3) as xtpool,
            tc.tile_pool(name="h1", bufs=2) as h1pool,
            tc.tile_pool(name="h2", bufs=4) as h2pool,
            tc.tile_pool(name="ps", bufs=8, space="PSUM") as pspool,
        ):
            # --- PE warm-up ---
            # The PE clock sits at 1.2GHz (HAM-throttled) until ~3.4us of
            # sustained activity. Burn that window on dummy matmuls over a
            # zeroed scratch tile while the startup DMAs are in flight, so
            # the real matmuls run at 2.4GHz from the first one.
            warm_sb = cpool.tile([P, N], FP16)
            nc.gpsimd.memset(warm_sb[:], 0.0)
            for i in range(7):
                wps = pspool.tile([P, N], FP32, tag="ps", name=f"warm{i}")
                nc.tensor.matmul(wps[:], warm_sb[:, 0:P], warm_sb[:], start=True, stop=True)

            # --- persistent SBUF state ---
            # startup-critical DMAs first: the sync sequencer issues one
            # DIRECT2D per ~0.6us, so issue order = time order. Interleave
            # per-k parts of xt[0] and w1 so the first matmuls can begin
            # after ~400KB instead of ~4MB; everything else queues behind.
            w1_sb = cpool.tile([P, KK1, 2, D_H], FP8)
            xt0_sb = xtpool.tile([P, KK1, 2, N], FP8, tag="xt", name="xt0")
            xt1_sb = xtpool.tile([P, KK1, 2, N], FP8, tag="xt", name="xt1")
            for kk in range(KK1):
                nc.sync.dma_start(xt0_sb[:, kk], xt_d[0, :, kk])
                # halves: the first phi1 matmuls need only m<4 of w1[kk]
                nc.sync.dma_start(w1_sb[:, kk, :, : D_H // 2], w1_d[:, kk, :, : D_H // 2])
                nc.sync.dma_start(w1_sb[:, kk, :, D_H // 2 :], w1_d[:, kk, :, D_H // 2 :])
            w2_sb = cpool.tile([P, KK2, 2, D_H], FP8)
            nc.sync.dma_start(w2_sb[:, : KK2 // 2], w2_d[:, : KK2 // 2])
            nc.sync.dma_start(w2_sb[:, KK2 // 2 :], w2_d[:, KK2 // 2 :])
            b1_sb = cpool.tile([P, K2], FP32)
            nc.sync.dma_start(b1_sb[:], b1_d)
            b2_sb = cpool.tile([P, K2], FP32)
            nc.sync.dma_start(b2_sb[:], b2_d)
            nc.sync.dma_start(xt1_sb[:], xt_d[1])
            # rho weights stream during the phi phase (issued early, consumed
            # by the rho1 passes that interleave into sample 7's phi2 slots)
            wr1_sb = cpool.tile([P, KR1, D_H], FP16)
            nc.sync.dma_start(wr1_sb[:], wr1_d)
            wr2_sb = cpool.tile([P, K2, D_H], FP16)
            nc.sync.dma_start(wr2_sb[:], wr2_d)
            br1_sb = cpool.tile([P, K2], FP32)
            nc.sync.dma_start(br1_sb[:], br1_d)
            br2_sb = cpool.tile([P, K2], FP32)
            nc.sync.dma_start(br2_sb[:], br2_d)

            pooled = cpool.tile([P, KR1, BL], FP32)  # [0:K2]=sum, [K2:]=raw max
            pooled_bf = cpool.tile([P, KR1, BL], FP16)
            r1_sb = cpool.tile([P, K2, BL], FP16)
            out_sb = cpool.tile([P, K2, BL], FP32)

            def phi1_mm(ps, m, kk, xt_sb, start, stop):
                # fp8 DoubleRow: lhsT [128, 2, 128], rhs [128, 2, 512];
                # contracts 256 input-feature rows per pass.
                nc.tensor.matmul(
                    ps[:],
                    w1_sb[:, kk, :, m * P : (m + 1) * P],
                    xt_sb[:, kk],
                    perf_mode=DOUBLE_ROW,
                    start=start,
                    stop=stop,
                )

            def phi1_evac(h1_sb, b, m, ps):
                # relu(psum + bias) -> fp8 h1 tile. GPSIMD can't read PSUM,
                # so this work is split between ScalarE and VectorE at the
                # ratio that balances their phi-phase load (ACT also carries
                # the phi2 sum-epilogue, DVE the max-reduces).
                if (b * K2 + m) % 3 == 0:
                    nc.scalar.activation(
                        h1_sb[:, m, :], ps[:], RELU,
                        bias=b1_sb[:, m : m + 1], scale=1.0,
                    )
                else:
                    nc.vector.tensor_scalar(
                        h1_sb[:, m, :], ps[:],
                        b1_sb[:, m : m + 1], 0.0,
                        OP_ADD, OP_MAX,
                    )

            def phi1(b):
                assert b == 0
                xt_sb = xt0_sb
                h1_sb = h1pool.tile([P, K2, N], FP8, tag="h1", name=f"h1_{b}")
                # two half-k accumulations across all m so the first 8
                # matmuls only need the first halves of the xt0/w1 DMAs.
                ps1 = []
                for m in range(K2):
                    ps = pspool.tile([P, N], FP32, tag="ps", name=f"ps1_0_{m}")
                    ps1.append(ps)
                    phi1_mm(ps, m, 0, xt_sb, start=True, stop=False)
                for m in range(K2):
                    ps = ps1[m]
                    phi1_mm(ps, m, 1, xt_sb, start=False, stop=True)
                    phi1_evac(h1_sb, 0, m, ps)
                return h1_sb

            def phi1_tile(b, m, xt_sb, h1_sb):
                ps = pspool.tile([P, N], FP32, tag="ps", name=f"ps1_{b}_{m}")
                for kk in range(KK1):
                    phi1_mm(ps, m, kk, xt_sb, start=(kk == 0), stop=(kk == KK1 - 1))
                phi1_evac(h1_sb, b, m, ps)

            def phi2_tile(b, m, h1_sb):
                    ps = pspool.tile([P, N], FP32, tag="ps", name=f"ps2_{b}_{m}")
                    for kk in range(KK2):
                        # fp8 DoubleRow over h1: rhs is two adjacent feature
                        # tiles [128, 2, 512] straight out of h1's layout.
                        nc.tensor.matmul(
                            ps[:],
                            w2_sb[:, kk, :, m * P : (m + 1) * P],
                            h1_sb[:, 2 * kk : 2 * kk + 2, :],
                            perf_mode=DOUBLE_ROW,
                            start=(kk == 0),
                            stop=(kk == KK2 - 1),
                        )
                    # relu(psum + bias) -> fp16 h2; sum over the set dim
                    # lands in pooled[:, m, b] via the activation accumulator.
                    h2_sb = h2pool.tile([P, N], FP16, tag="h2", name=f"h2_{b}_{m}")
                    nc.scalar.activation(
                        h2_sb[:],
                        ps[:],
                        RELU,
                        bias=b2_sb[:, m : m + 1],
                        scale=1.0,
                        accum_out=pooled[:, m, b : b + 1],
                    )
                    if b == BL - 1:
                        # last sample: the sum feature tile is complete as soon
                        # as the ACT accumulator lands -> cast it (in-order on
                        # ACT) so rho1's sum-half matmuls can start.
                        nc.scalar.copy(pooled_bf[:, m, :], pooled[:, m, :])
                    # max-pool over the set dim from the fp16 h2 (SBUF reads
                    # are a DVE access-tier cheaper than PSUM reads).
                    nc.vector.tensor_reduce(
                        pooled[:, K2 + m, b : b + 1], h2_sb[:], axis=AX_X, op=OP_MAX
                    )
                    if b == BL - 1:
                        # in-order on DVE right after its own max-reduce
                        nc.vector.tensor_copy(pooled_bf[:, K2 + m, :], pooled[:, K2 + m, :])

            # software pipeline, interleaved at m-tile granularity: each slot
            # emits phi2(b-1, m) then phi1(b, m), so at most ~4 PSUM tiles are
            # in flight (vs 16 for whole-sample pipelining) and the ACT/DVE
            # epilogue load is spread evenly in time.
            prev_h1 = phi1(0)
            for b in range(1, BL):
                if b == 1:
                    xt_sb = xt1_sb
                else:
                    xt_sb = xtpool.tile([P, KK1, 2, N], FP8, tag="xt", name=f"xt{b}")
                    nc.sync.dma_start(xt_sb[:], xt_d[b])
                h1_cur = h1pool.tile([P, K2, N], FP8, tag="h1", name=f"h1_{b}")
                for m in range(K2):
                    phi2_tile(b - 1, m, prev_h1)
                    phi1_tile(b, m, xt_sb, h1_cur)
                prev_h1 = h1_cur

            # --- rho MLP, pipelined into sample 7's phi2 ---
            # PSUM accumulation groups need a bank each (a start=True matmul
            # resets its whole bank), so rho1 runs as two sets of 4 out-tiles
            # with their own pool tiles. The first set's k-passes interleave
            # into sample 7's phi2 slots (pooled tile k is consumed the moment
            # its cast lands); the second set chases the epilogue drain.
            MH = K2 // 2
            psr1 = {}

            def rho1_pass(ms, k, start, stop):
                for m in ms:
                    if m not in psr1:
                        psr1[m] = pspool.tile([P, BL], FP32, tag="ps", name=f"psr1_{m}")
                    nc.tensor.matmul(
                        psr1[m][:],
                        wr1_sb[:, k, m * P : (m + 1) * P],
                        pooled_bf[:, k, :],
                        start=start,
                        stop=stop,
                    )

            lo = range(MH)
            hi = range(MH, K2)
            for m in range(K2):
                phi2_tile(BL - 1, m, prev_h1)
                rho1_pass(lo, m, start=(m == 0), stop=False)  # sum tile m ready
                if m >= 1:
                    rho1_pass(lo, K2 + m - 1, start=False, stop=False)
            for k in range(KR1 - 1):
                rho1_pass(hi, k, start=(k == 0), stop=False)
            rho1_pass(lo, KR1 - 1, start=False, stop=True)
            rho1_pass(hi, KR1 - 1, start=False, stop=True)

            for m in range(K2):
                # DVE's small-tile tensor_scalar (~220ns) beats ACT's ~600ns
                # ACTIVATE here, and rho2's k-chain waits on these in order.
                nc.vector.tensor_scalar(
                    r1_sb[:, m, :], psr1[m][:],
                    br1_sb[:, m : m + 1], 0.0,
                    OP_ADD, OP_MAX,
                )
            for m in range(K2):
                ps = pspool.tile([P, BL], FP32, tag="ps", name=f"psr2_{m}")
                for k in range(K2):
                    nc.tensor.matmul(
                        ps[:],
                        wr2_sb[:, k, m * P : (m + 1) * P],
                        r1_sb[:, k, :],
                        start=(k == 0),
                        stop=(k == K2 - 1),
                    )
                nc.vector.tensor_scalar(
                    out_sb[:, m, :], ps[:],
                    br2_sb[:, m : m + 1], 0.0,
                    OP_ADD, OP_MAX,
                )
                if m == K2 // 2 - 1:
                    # first half of the output leaves while rho2 finishes
                    nc.sync.dma_start(out_d[:, : K2 // 2], out_sb[:, : K2 // 2])
            nc.sync.dma_start(out_d[:, K2 // 2 :], out_sb[:, K2 // 2 :])

    return nc


_CACHE: dict = {}


def get_compiled() -> bacc.Bacc:
    if "nc" not in _CACHE:
        nc = build_program()
        nc.compile()
        _CACHE["nc"] = nc
    return _CACHE["nc"]


# e4m3 grid for the error-diffusing quantizer (finite values, sorted)
_E4M3_GRID = np.unique(
    np.arange(256, dtype=np.uint8).view(ml_dtypes.float8_e4m3).astype(np.float32)
)
_E4M3_GRID = _E4M3_GRID[np.isfinite(_E4M3_GRID)]


def _err_diffuse_fp8(W: np.ndarray, mu: np.ndarray) -> np.ndarray:
    """Quantize W [fan_in, fan_out] to e4m3, diffusing the mu-weighted
    rounding error down each column: every weight rounds to the fp8 grid
    neighbor that keeps |sum_i mu[i] * err[i, h]| minimal. This cancels the
    coherent error term (mu . dW) that sum-pooling would otherwise amplify."""
    W = np.asarray(W, np.float32)
    idx = np.searchsorted(_E4M3_GRID, W)
    lo = _E4M3_GRID[np.clip(idx - 1, 0, len(_E4M3_GRID) - 1)]
    hi = _E4M3_GRID[np.clip(idx, 0, len(_E4M3_GRID) - 1)]
    e_lo = lo - W
    e_hi = hi - W
    Wq = np.empty_like(W)
    S = np.zeros(W.shape[1], np.float64)
    for i in range(W.shape[0]):
        c_lo = np.abs(S + mu[i] * e_lo[i])
        c_hi = np.abs(S + mu[i] * e_hi[i])
        use_lo = c_lo <= c_hi
        Wq[i] = np.where(use_lo, lo[i], hi[i])
        S += mu[i] * np.where(use_lo, e_lo[i], e_hi[i])
    return Wq.astype(NP_FP8)


def stage_inputs(x, W_phi1, b_phi1, W_phi2, b_phi2, W_rho1, b_rho1, W_rho2, b_rho2):
    """Host-side staging: transpose x, quantize weights, reshape biases."""

    def wtile(a):
        # [KO*P, H] -> [P, KO, H] with w[p, ko, h] = W[ko*P + p, h]
        a = np.asarray(a, np.float32).astype(NP_FP16)
        ko = a.shape[0] // P
        return np.ascontiguousarray(a.reshape(ko, P, -1).transpose(1, 0, 2))

    def bias(a):
        # [n_tiles*P] -> [P, n_tiles] with b_sb[p, m] = b[m*P + p]
        return np.ascontiguousarray(np.asarray(a, np.float32).reshape(-1, P).T)

    # x[b, n, d] -> xt[b, p, kk, j, n] = x[b, n, kk*256 + j*128 + p]  (fp8)
    xt = np.asarray(x, np.float32).astype(NP_FP8)
    xt = np.ascontiguousarray(xt.reshape(B, N, KK1, 2, P).transpose(0, 4, 2, 3, 1))
    # W1[d, h] -> w1[p, kk, j, h] = W1[kk*256 + j*128 + p, h]  (fp8)
    w1 = np.asarray(W_phi1, np.float32).astype(NP_FP8)
    w1t = np.ascontiguousarray(w1.reshape(KK1, 2, P, D_H).transpose(2, 0, 1, 3))
    # mu = E[h1] under x ~ N(0, I): h1_i = relu(N(b1_i, ||W1q[:, i]||^2)).
    sig = np.linalg.norm(w1.astype(np.float32), axis=0)
    z = np.asarray(b_phi1, np.float32) / np.maximum(sig, 1e-12)
    pdf = np.exp(-0.5 * z * z) / math.sqrt(2 * math.pi)
    cdf = 0.5 * (1.0 + np.vectorize(math.erf)(z / math.sqrt(2.0)))
    mu = sig * pdf + np.asarray(b_phi1, np.float32) * cdf
    # W2 error-diffused to fp8, then w2[p, kk, j, h] = W2q[kk*256 + j*128 + p, h]
    w2 = _err_diffuse_fp8(np.asarray(W_phi2, np.float32), mu)
    w2t = np.ascontiguousarray(w2.reshape(KK2, 2, P, D_H).transpose(2, 0, 1, 3))
    shared = {
        "w1": w1t,
        "w2": w2t,
        "wr1": wtile(W_rho1),
        "wr2": wtile(W_rho2),
        "b1": bias(b_phi1),
        "b2": bias(b_phi2),
        "br1": bias(b_rho1),
        "br2": bias(b_rho2),
    }
    in_maps = []
    for c in range(N_CORES):
        m = dict(shared)
        m["xt"] = np.ascontiguousarray(xt[c * BL : (c + 1) * BL])
        in_maps.append(m)
    return in_maps


def gather_output(results) -> np.ndarray:
    # per-core out: [P, K2, BL] with out[p, m, s] = r2[m*128+p, s]
    parts = []
    for c in range(N_CORES):
        o = np.asarray(results[c]["out"], np.float32)  # [P, K2, BL]
        parts.append(o.transpose(2, 1, 0).reshape(BL, D_H))  # [BL, D_H]
    return np.concatenate(parts, axis=0)


def run(trace: bool = False, **inputs):
    nc = get_compiled()
    in_maps = stage_inputs(**inputs)
    res = run_bass_kernel_spmd(nc, in_maps, core_ids=list(range(N_CORES)), trace=trace)
    return gather_output(res.results), res


def kernel(**inputs) -> np.ndarray:
    out, _ = run(trace=False, **inputs)
    return out
